# revision 42
# baseline (speedup 1.0000x reference)
"""Trainium2 Bass kernel for nn_Attention_28930899706081 (sparse_attention).

Reference computation:
  k1 = l2norm_c(Wqk @ fmap1), k2 = l2norm_c(Wqk @ fmap2), q = l2norm_c(Wqk @ dmap)
  sim_i = q^T k_i per batch  -> [b, n, n] with n = h*w = 4096
  attn_i = softmax(sim_i, axis=-1)[:, None]  -> [b, 1, n, n]

Sharding: 8 cores; core i handles batch b = i//4 and query-row block r = i%4
(1024 of 4096 rows). Each core computes the full normalized K for its batch
and its row block of both sims + softmax.

v2 design (from trace analysis of the 116us baseline — ScalarE was the
bottleneck at ~100us busy):
  - inputs shipped as fp8e4m3 (w pre-scaled x16; any common scale cancels in
    the L2 normalization), halving input DMA and leaving projection error
    ~3e-3 on attn, well under the 2e-2 gate.
  - squares for the column norms on the Pool engine (gpsimd) instead of ACT.
  - softmax exp split between ACT (true exp, free accumulator row-sums) and
    a custom DVE op EXP2SQ_REDUCE_ANT: exp(x) ~= (1 + x(b1 + x(b2 + x b3)))^2
    valid on |x|<=1.1 (unit-vector dot products), with fused row-sum accum.
    Assignment alternates per 128-row tile.
  - final e*recip muls alternate DVE / Pool per row tile.
  - output written as one 1MB DMA per row tile (16 total) on the sync queue.
"""

import numpy as np
import ml_dtypes

B, C, H, W, D = 2, 256, 64, 64, 128
N = H * W  # 4096
QBLK = N // 4  # 1024 query rows per core
N_CORES = 8

# exp(x) ~= (1 + x(B1 + x(B2 + x B3)))^2, max rel err 2.4e-3 on [-1.1, 1.1]
EXP_B1, EXP_B2, EXP_B3 = 0.50085628, 0.12743257, 0.02029121

_cached = {}


def _register_exp_op():
    """Register the custom DVE op: out = (1 + x(b1 + x(b2 + x b3)))^2,
    accum_out = row sums. 8 ALU stages (at the limit), 1 elem/cycle/lane."""
    from operator import add
    import concourse.dve_ops as dve_ops
    from concourse.dve_spec import Spec, Src0, C0, C1, C2, One, sq, lower
    from concourse.dve_uop import DveOpSpec

    name = "EXP2SQ_REDUCE_ANT"
    for o in dve_ops.OPS:
        if o.name == name:
            return o

    body = sq(One + Src0 * (C0 + Src0 * (C1 + Src0 * C2)))

    def ref(in0, in1, s0, s1, imm2):
        p = 1.0 + in0 * (s0 + in0 * (s1 + in0 * imm2))
        b = (p * p).astype(np.float32)
        return b, b.reshape(b.shape[0], -1).sum(axis=-1, keepdims=True)

    spec = Spec(body=body, accum=add, reference=ref)
    row = max(dve_ops._SUB_OPCODE_FOR_NAME.values()) + 1
    shas = {}
    for ver in ("v3", "v4"):
        try:
            uops = lower(spec, ver=ver)
            shas[ver] = DveOpSpec(
                name=name, opcode=row, uops=uops, rd1_en=False
            ).sha(ver)
        except Exception:
            pass
    op = dve_ops.DveOp(name, spec, subdim=False, uops_sha=shas)
    dve_ops.OPS.append(op)
    dve_ops.CUSTOM_DVE_SPECS[name] = spec
    dve_ops._SUB_OPCODE_FOR_NAME[name] = row
    return op


def _build():
    import concourse.mybir as mybir
    import concourse.tile as tile
    from concourse import bacc
    from contextlib import ExitStack

    exp_op = _register_exp_op()

    f32 = mybir.dt.float32
    f16 = mybir.dt.float16
    bf16 = mybir.dt.bfloat16
    fp8 = mybir.dt.float8e4
    AF = mybir.ActivationFunctionType

    nc = bacc.Bacc(
        "TRN2",
        target_bir_lowering=False,
        debug=False,
        enable_asserts=False,
        num_devices=N_CORES,
    )

    f1_ext = nc.dram_tensor("f1", [C, N], fp8, kind="ExternalInput").ap()
    f2_ext = nc.dram_tensor("f2", [C, N], fp8, kind="ExternalInput").ap()
    xq_ext = nc.dram_tensor("xq", [C, QBLK], fp8, kind="ExternalInput").ap()
    wqkT_ext = nc.dram_tensor("wqkT", [C, D], fp8, kind="ExternalInput").ap()
    out_ext = nc.dram_tensor("out", [2, QBLK, N], bf16, kind="ExternalOutput").ap()

    PCH = 512  # matmul free-dim chunk (one PSUM bank)
    CH = 2048  # softmax pipeline chunk
    XCH = 1024  # phase A chunk

    with tile.TileContext(nc) as tc, ExitStack() as ctx:
        consts = ctx.enter_context(tc.tile_pool(name="consts", bufs=1))
        xq_in = ctx.enter_context(tc.tile_pool(name="xqin", bufs=2))
        xk_in = ctx.enter_context(tc.tile_pool(name="xkin", bufs=4))
        ysq_pool = ctx.enter_context(tc.tile_pool(name="ysq", bufs=10))
        rk_pool = ctx.enter_context(tc.tile_pool(name="rk", bufs=10))
        kn_pool = ctx.enter_context(tc.tile_pool(name="kn", bufs=1))
        e_pool = ctx.enter_context(tc.tile_pool(name="epool", bufs=6))
        attn_pool = ctx.enter_context(tc.tile_pool(name="attn", bufs=4))
        lnt_pool = ctx.enter_context(tc.tile_pool(name="lnt", bufs=3))
        stat_pool = ctx.enter_context(tc.tile_pool(name="stat", bufs=8))

        wqkT_sb = [
            consts.tile([128, D], fp8, tag=f"wqkT{k}", name=f"wqkT{k}")
            for k in range(2)
        ]
        nc.sync.dma_start(out=wqkT_sb[0][:], in_=wqkT_ext[0:128, :])
        nc.sync.dma_start(out=wqkT_sb[1][:], in_=wqkT_ext[128:256, :])
        ones_sb = consts.tile([128, 128], bf16, tag="ones", name="ones")
        nc.vector.memset(ones_sb[:], 1.0)
        # dummy Ln first: loads natural_log_exp_and_others during the DMA
        # lead-in. That set holds BOTH ln and exp, so the whole kernel runs
        # on ONE table set: rsqrt becomes exp(-0.5*ln(x)) and the mid-kernel
        # exp-table load + drain right before the first sim exp disappears.
        warm = consts.tile([128, 1], f32, tag="warm", name="warm")
        nc.scalar.activation(out=warm[:], in_=ones_sb[:, 0:1], func=AF.Ln)

        with tc.tile_pool(name="proj_psum", bufs=2, space="PSUM") as proj_psum, \
             tc.tile_pool(name="n2_psum", bufs=2, space="PSUM") as n2_psum:

            # Deferred normalize-muls: the xn mul depends on the ACT rsqrt;
            # emitting it inline stalls the in-order DVE queue (cost ~23us in
            # the v3 trace). Defer each mul two chunks behind instead.
            pending = []
            chunk_idx = [0]

            def flush_pending(keep):
                while len(pending) > keep:
                    xn_sl, y_bf, rk = pending.pop(0)
                    nc.vector.tensor_mul(xn_sl, y_bf[:], rk[:])

            def phase_a(x_ext, ncols, tagbase, in_pool):
                """DMA + project + l2-normalize columns, chunk-pipelined.
                matmuls on PE, copy+square+scale on DVE (16-bit 2x where
                possible), rsqrt on ACT."""
                xn = kn_pool.tile([128, ncols], bf16, tag=tagbase, name=tagbase)
                x_lo = in_pool.tile([128, ncols], fp8, tag="xlo", name="x_lo")
                x_hi = in_pool.tile([128, ncols], fp8, tag="xhi", name="x_hi")
                nc.sync.dma_start(out=x_lo[:], in_=x_ext[0:128, :])
                nc.sync.dma_start(out=x_hi[:], in_=x_ext[128:256, :])
                for h in range(ncols // XCH):
                    h0 = h * XCH
                    ps = proj_psum.tile([128, XCH], f32, tag="proj", name="pps")
                    for c in range(XCH // PCH):
                        sl = slice(h0 + c * PCH, h0 + (c + 1) * PCH)
                        psl = ps[:, c * PCH : (c + 1) * PCH]
                        nc.tensor.matmul(
                            psl, wqkT_sb[0][:], x_lo[:, sl], start=True, stop=False
                        )
                        nc.tensor.matmul(
                            psl, wqkT_sb[1][:], x_hi[:, sl], start=False, stop=True
                        )
                    y_bf = ysq_pool.tile([128, XCH], bf16, tag="ybf", name="y_bf")
                    nc.vector.tensor_copy(y_bf[:], ps[:])
                    ysq = ysq_pool.tile([128, XCH], bf16, tag="ysq", name="ysq")
                    nc.vector.tensor_mul(ysq[:], y_bf[:], y_bf[:])
                    nps = n2_psum.tile([128, XCH], f32, tag="n2", name="nps")
                    for c in range(XCH // PCH):
                        nc.tensor.matmul(
                            nps[:, c * PCH : (c + 1) * PCH],
                            ones_sb[:],
                            ysq[:, c * PCH : (c + 1) * PCH],
                            start=True,
                            stop=True,
                        )
                    lnt = lnt_pool.tile([128, XCH], f32, tag="lnt", name="lnt")
                    nc.scalar.activation(out=lnt[:], in_=nps[:], func=AF.Ln)
                    rk = rk_pool.tile([128, XCH], bf16, tag="rk", name="rk")
                    nc.scalar.activation(
                        out=rk[:], in_=lnt[:], func=AF.Exp, scale=-0.5
                    )
                    pending.append((xn[:, h0 : h0 + XCH], y_bf, rk))
                return xn

            qn = phase_a(xq_ext, QBLK, "qn", xq_in)
            k1n = phase_a(f1_ext, N, "k1n", xk_in)
            k2n = phase_a(f2_ext, N, "k2n", xk_in)
            flush_pending(0)

        with tc.tile_pool(name="sim_psum", bufs=2, space="PSUM") as sim_psum:
            # chunk j=0 always on ACT; j=1 on DVE for 10 of 16 row tiles.
            # Fine interleave keeps both exp engines continuously fed (a
            # whole-tile alternation left 2.5-3.3us ACT gaps per DVE tile).
            DVE_TILES = frozenset((0, 2, 3, 5, 6, 8, 9, 11, 12, 14))

            def phase_b(kn, s):
                """row block of sim + softmax for one K map, streamed to out[s]."""
                for t in range(QBLK // 128):
                    g = s * (QBLK // 128) + t  # global row-tile index 0..15
                    lhsT = qn[:, t * 128 : (t + 1) * 128]
                    attn = attn_pool.tile([128, N], bf16, tag="attn", name="attn")
                    stA = stat_pool.tile([128, 1], f32, tag="stA", name="stA")
                    stB = stat_pool.tile([128, 1], f32, tag="stB", name="stB")
                    e_chunks = []
                    for j in range(N // CH):
                        ps = sim_psum.tile([128, CH], f32, tag="sim", name="sim_ps")
                        for c in range(CH // PCH):
                            csl = slice(j * CH + c * PCH, j * CH + (c + 1) * PCH)
                            nc.tensor.matmul(
                                ps[:, c * PCH : (c + 1) * PCH],
                                lhsT,
                                kn[:, csl],
                                start=True,
                                stop=True,
                            )
                        e = e_pool.tile([128, CH], bf16, tag="e", name="e")
                        acc = (stA if j == 0 else stB)[:]
                        if j == 1 and g in DVE_TILES:
                            nc.vector._custom_dve(
                                exp_op,
                                out=e[:],
                                in0=ps[:],
                                s0=EXP_B1,
                                s1=EXP_B2,
                                imm2=EXP_B3,
                                accum_out=acc,
                            )
                        else:
                            nc.scalar.activation(
                                out=e[:],
                                in_=ps[:],
                                func=AF.Exp,
                                accum_out=acc,
                            )
                        e_chunks.append(e)
                    ssum = stat_pool.tile([128, 1], f32, tag="ssum", name="ssum")
                    nc.vector.tensor_add(ssum[:], stA[:], stB[:])
                    recip = stat_pool.tile([128, 1], f32, tag="recip", name="recip")
                    nc.vector.reciprocal(recip[:], ssum[:])
                    for j, e in enumerate(e_chunks):
                        nc.vector.tensor_scalar_mul(
                            attn[:, j * CH : (j + 1) * CH], e[:], recip[:]
                        )
                        nc.sync.dma_start(
                            out=out_ext[
                                s, t * 128 : (t + 1) * 128, j * CH : (j + 1) * CH
                            ],
                            in_=attn[:, j * CH : (j + 1) * CH],
                        )

            phase_b(k1n, 0)
            phase_b(k2n, 1)

    nc.compile()
    return nc


def _get_nc():
    if "nc" not in _cached:
        _cached["nc"] = _build()
    return _cached["nc"]


def _in_maps(fmap1, fmap2, dmap, Wqk):
    f8 = ml_dtypes.float8_e4m3
    f1r = np.asarray(fmap1, dtype=np.float32).reshape(B, C, N)
    f2r = np.asarray(fmap2, dtype=np.float32).reshape(B, C, N)
    dqr = np.asarray(dmap, dtype=np.float32).reshape(B, C, N)
    # x16: keeps the fp8 weights out of the subnormal range; the common
    # scale cancels exactly in the L2 normalization.
    wT = np.ascontiguousarray(np.asarray(Wqk, dtype=np.float32).T * 16.0).astype(f8)

    in_maps = []
    for i in range(N_CORES):
        b, r = divmod(i, 4)
        in_maps.append(
            {
                "f1": np.ascontiguousarray(f1r[b]).astype(f8),
                "f2": np.ascontiguousarray(f2r[b]).astype(f8),
                "xq": np.ascontiguousarray(
                    dqr[b][:, r * QBLK : (r + 1) * QBLK]
                ).astype(f8),
                "wqkT": wT,
            }
        )
    return in_maps


def kernel(fmap1, fmap2, dmap, Wqk):
    from concourse.bass_utils import run_bass_kernel_spmd

    in_maps = _in_maps(fmap1, fmap2, dmap, Wqk)
    nc = _get_nc()
    res = run_bass_kernel_spmd(nc, in_maps, core_ids=list(range(N_CORES)))
    _cached["last_result"] = res

    attn1 = np.empty((B, 1, N, N), dtype=np.float32)
    attn2 = np.empty((B, 1, N, N), dtype=np.float32)
    for i in range(N_CORES):
        b, r = divmod(i, 4)
        o = res.results[i]["out"]
        attn1[b, 0, r * QBLK : (r + 1) * QBLK, :] = o[0].astype(np.float32)
        attn2[b, 0, r * QBLK : (r + 1) * QBLK, :] = o[1].astype(np.float32)
    return (attn1, attn2)
